# revision 51
# baseline (speedup 1.0000x reference)
"""Trainium2 Bass kernel for AdaptiveNet MLP (fc1+sigmoid, grouped fc2+sigmoid, fc3).

Sharding: pure data-parallel over batch across 8 NeuronCores (no collectives).
Each core computes its 2048-row shard through all three layers.

fc1 (95% of FLOPs) runs in fp8-e4m3 with DoubleRow perf mode (two fp8 weights
per PE cell -> K=256 per matmul, halving the matmul count); the sigmoid damps
the quantization error so the final rel-err stays ~3e-3 (gate is 2e-2).

Layout trick: H1 is permuted s-major on the host (h1' = s*512 + g, where the
original h1 = g*8 + s).  fc1 then produces hT' tiles [128 h1' partitions x 512
rows]; the grouped fc2 contraction over s becomes 8 fused multiply-accumulate
ops on the vector engine with per-partition scalars (W2 columns), and fc3 is a
plain bf16 matmul over the 512 groups.  Biases are per-partition [128,1]
columns fused into ScalarE sigmoids / a VectorE add.

The PE roofline for this shape is ~110.6us of fc1 matmuls (N=512 columns at
1 col/cycle @2.4GHz, fp8-DR K=256/pass — measured: every N=512 matmul costs
216ns regardless of dtype/perf-mode) + ~7us fc3, behind a fixed ~7.1us NEFF
preamble.  Overlap work to sit on that roofline:

- PE warmup: 14 throwaway DR matmuls on a Pool-memset scratch tile keep the
  PE busy from ~7.7us until the first real slices land (~11us); a PE idle
  before the DVFS ramp completes costs ~1-2us of 1.2GHz matmuls.
- DMA head: the first ~22us is bandwidth-limited (~220GB/s effective).  All
  of group0's phases run j-outer and x_0..x_3 arrive as per-j-pair 128KB
  slices in exact consumption order.  Lane rules learned from traces: the
  sync HWDGE ring is the only fast multi-DMA queue; the scalar ring delivers
  its 2nd+ DMA ~4us late (gets only x_0 j01); gpsimd SWDGE takes max 4 DMAs
  (a 5th DRAIN-blocks its ring ~10us).
- fc2 chain scheduling: the last 16 chunks are grouped [16,20,24,28,19],
  [17,21,25,29,23], [18,22,26,30,27], [31] so chains t0..t2 complete inside
  4.3us phases with ACT slack, and the end-of-kernel chain for t3 is a
  single sigmoid+STT+sigmoid.
- PSUM: fc1 chunks cycle a 5-bank pool, fc3 po tiles a separate 3-bank pool
  (sharing one pool put fc3(1)'s banks behind the final sigmoid chain).
- Tail: fc3 runs t-outer with the final chunk half-width; fc3(1) is
  deferred last so its t0..t2 matmuls cover the final sigmoid chain; the
  last block's bias+output run as halves pipelined with the t3 matmuls,
  d0 on DVE+sync and d1 on ACT(Identity+bias)+scalar.
"""

import sys

for _p in ("/opt/trn_rl_repo",):
    if _p not in sys.path:
        sys.path.append(_p)

import numpy as np
import ml_dtypes

BF16 = ml_dtypes.bfloat16
FP8 = ml_dtypes.float8_e4m3  # == mybir.dt.float8e4

D_IN, H1, H2, D_OUT = 1024, 4096, 512, 256
GS = H1 // H2  # 8
B = 16384
N_CORES = 8
B_SHARD = B // N_CORES  # 2048
NBLK = 512  # rows per block (one PSUM bank of fp32)
NB = B_SHARD // NBLK  # 4
KC = D_IN // 128  # 8 contraction subtiles for fc1
KP = KC // 2  # 4 DoubleRow pairs
CC = H1 // 128  # 32 h1' chunks
NT = H2 // 128  # 4 x2T tiles
ND = D_OUT // 128  # 2 output chunks

_compiled = {}


def _build_nc():
    from concourse import bacc, tile, mybir

    f32 = mybir.dt.float32
    bf16 = mybir.dt.bfloat16
    fp8 = mybir.dt.float8e4
    AF = mybir.ActivationFunctionType
    ALU = mybir.AluOpType
    DR = mybir.MatmulPerfMode.DoubleRow

    nc = bacc.Bacc("TRN2", target_bir_lowering=False, debug=False,
                   num_devices=N_CORES)

    xq = nc.dram_tensor("xq", [128, KC, B_SHARD], fp8, kind="ExternalInput")
    w1q = nc.dram_tensor("w1q", [128, KC, H1], fp8, kind="ExternalInput")
    # all [128, *] f32 constants packed on the free axis:
    # b1 (CC) | w2 (CC) | b2 (NT) | b3 (ND)
    cst = nc.dram_tensor("cst", [128, 2 * CC + NT + ND], f32,
                         kind="ExternalInput")
    # W3' = 2*W3 in fp8, DoubleRow pair-interleaved: [p, jj, e, m] holds
    # w3t'[128*(2jj+e)+p, m]
    w3q = nc.dram_tensor("w3q", [128, 2, 2, D_OUT], fp8, kind="ExternalInput")
    out = nc.dram_tensor("out", [D_OUT, B_SHARD], bf16, kind="ExternalOutput")

    with tile.TileContext(nc) as tc:
        with (
            tc.tile_pool(name="wpool", bufs=1) as wpool,
            tc.tile_pool(name="xpool", bufs=1) as xpool,
            tc.tile_pool(name="hpool", bufs=8) as hpool,
            tc.tile_pool(name="accpool", bufs=1) as accpool,
            tc.tile_pool(name="x2pool", bufs=1) as x2pool,
            tc.tile_pool(name="opool", bufs=4) as opool,
            tc.tile_pool(name="psum_h", bufs=5, space="PSUM") as psum_h_pool,
            tc.tile_pool(name="psum_o", bufs=3, space="PSUM") as psum_o_pool,
        ):
            # fc3 po tiles get their own 3-bank pool: sharing all 8 banks
            # with fc1 made fc3(1)'s po reuse the bank of block1's final
            # ph chunk, which frees only after the last sigmoid chain ->
            # ~1.4us PE stall right before the last matmuls
            w1_sb = wpool.tile([128, KC, H1], fp8, tag="w1")
            x_sb = [None] * NB
            for n in range(NB):
                x_sb[n] = xpool.tile([128, KC, NBLK], fp8,
                                     tag=f"x_{n}", name=f"xsb_{n}")
            # Four issuing queues (sync/scalar HWDGE ~0.6us first-byte,
            # vector/gpsimd SWDGE ~1us), FIFO within each.  The NEFF preamble
            # (engine sem barrier + instruction-stream loads) runs to ~7.1us,
            # so the first dma_start issues ~7.2us and the first 32-128KB
            # slices land ~9.5-10us.  Slice the head so the very first
            # matmuls (block0, j-outer) unblock on 32KB granularity.
            CBLK = 4
            WBLK = 8

            cst_sb = wpool.tile([128, 2 * CC + NT + ND], f32, tag="cst")
            w3_sb = wpool.tile([128, 2, 2, D_OUT], fp8, tag="w3q")

            # --- PE warmup: keep the PE busy through the input-DMA head so
            # the HAM clock gate flips to 8/8 before real matmuls arrive and
            # the cold 1.2GHz window is spent on throwaway work.  The scratch
            # tile is memset on the Pool queue (first op, ~0.1us) so warmup
            # matmuls start ~7.4us; 10 N=256 DR matmuls cover until ~9.8us
            # when the first real slices land. ---
            wm = wpool.tile([128, 2, 256], fp8, tag="wm")
            nc.gpsimd.memset(wm[:], 0)
            wm_ps = psum_h_pool.tile([128, 256], f32, tag="psum_h",
                                     name="wm_ps")
            for _wi in range(14):
                nc.tensor.matmul(wm_ps[:], lhsT=wm[:, :, 0:128], rhs=wm[:],
                                 start=True, stop=True,
                                 perf_mode=DR)

            def b1col(c):
                return cst_sb[:, c:c + 1]

            def w2col(c):
                return cst_sb[:, CC + c:CC + c + 1]

            def b2col(t):
                return cst_sb[:, 2 * CC + t:2 * CC + t + 1]

            def b3col(d):
                return cst_sb[:, 2 * CC + NT + d:2 * CC + NT + d + 1]

            # one DMA per W1 column-block covering all subtile pairs; the
            # first block split in half so the very first matmuls unblock
            # sooner
            def wblock(cb0, cb1, eng):
                c0, c1 = cb0 * 128, cb1 * 128
                eng.dma_start(w1_sb[:, :, c0:c1], w1q.ap()[:, :, c0:c1])

            def xdma(n, eng):
                eng.dma_start(x_sb[n][:],
                              xq.ap()[:, :, n * NBLK:(n + 1) * NBLK])

            # Lane layout tuned so block0's j-outer stream is gapless from
            # first matmul (a PE idle before the clock ramp completes resets
            # it to 1.2GHz for ~4us).  Per-j-pair arrivals interleave across
            # sync/scalar (HWDGE) and gpsimd (SWDGE, max 4 DMAs or its ring
            # DRAIN-blocks for ~10us).
            # sync (fast HWDGE ring): all block0 W1 j-pair slices in
            # consumption order, then x_1/x_3/x_2, the big W1 blocks, w3.
            # The scalar ring delivers its 2nd+ DMA ~4us late, so it gets
            # only x_0 j01; gpsimd (SWDGE, max 4 DMAs) takes the other x_0
            # j-pair slices + the small consts.
            nc.sync.dma_start(w1_sb[:, 0:2, 0:CBLK * 128],
                              w1q.ap()[:, 0:2, 0:CBLK * 128])
            nc.sync.dma_start(w1_sb[:, 2:4, 0:CBLK * 128],
                              w1q.ap()[:, 2:4, 0:CBLK * 128])
            nc.sync.dma_start(w1_sb[:, 4:6, 0:CBLK * 128],
                              w1q.ap()[:, 4:6, 0:CBLK * 128])
            nc.sync.dma_start(w1_sb[:, 6:8, 0:CBLK * 128],
                              w1q.ap()[:, 6:8, 0:CBLK * 128])
            # x_1/x_3/x_2 per-j-pair slices in exact consumption order of
            # the j-outer group0 phases (NORD = 0,1,3,2)
            for nn in (1, 3, 2):
                for j in range(KP):
                    nc.sync.dma_start(
                        x_sb[nn][:, 2 * j:2 * j + 2, :],
                        xq.ap()[:, 2 * j:2 * j + 2,
                                nn * NBLK:(nn + 1) * NBLK])
            for cb in range(CBLK, CC, WBLK):
                wblock(cb, min(cb + WBLK, CC), nc.sync)
            nc.sync.dma_start(w3_sb[:], w3q.ap()[:])
            # scalar: x_0 j01 only (gates the first real matmul; this
            # queue's 2nd DMA arrives ~4us late, so it gets just one)
            nc.scalar.dma_start(x_sb[0][:, 0:2, :], xq.ap()[:, 0:2, 0:NBLK])
            # gpsimd: x_0 j23/j45/j67 + consts (exactly 4 DMAs; a 5th
            # DRAIN-blocks its SWDGE ring for ~10us)
            nc.gpsimd.dma_start(x_sb[0][:, 2:4, :], xq.ap()[:, 2:4, 0:NBLK])
            nc.gpsimd.dma_start(x_sb[0][:, 4:6, :], xq.ap()[:, 4:6, 0:NBLK])
            nc.gpsimd.dma_start(x_sb[0][:, 6:8, :], xq.ap()[:, 6:8, 0:NBLK])
            nc.gpsimd.dma_start(cst_sb[:], cst.ap()[:])

            # fc2 accumulators, one per (row-block, x2 tile)
            acc = [[None] * NT for _ in range(NB)]

            # x2 activations in fp8, paired per DoubleRow k-pair jj:
            # tile [128, 2, NBLK] holds tanh((z+b2)/2) for t=2jj (e=0) and
            # t=2jj+1 (e=1).  tanh((z+b2)/2) = sigmoid(z+b2) - 0.5 exactly;
            # the 0.5 shift and 2x tanh scale fold into b3'/W3' on the host
            # (b3' = b3 + 0.5*W3.sum(1), W3' = 2*W3, out = 0.25*psum + b3').
            # Centering halves the fp8 quantization error; measured total
            # rel-err 7.7e-3 vs the 2e-2 gate.
            x2_sb = [[None] * 2 for _ in range(NB)]

            # fc2 accumulate chains all on DVE: Pool (= the GPSIMD Q7
            # complex on TRN2) has no TensorScalarPtr opcode, is ~2x slower
            # for elementwise, and shares its SBUF port with DVE anyway
            def fc2_eng(t_i):
                return nc.vector

            def fc2_step(c, n, ht):
                t_i = c % NT
                if c < NT:
                    acc[n][t_i] = accpool.tile([128, NBLK], bf16,
                                               tag=f"acc_{n}_{t_i}",
                                               name=f"acc_{n}_{t_i}")
                    fc2_eng(t_i).tensor_scalar_mul(acc[n][t_i][:], ht[:],
                                                   w2col(c))
                else:
                    fc2_eng(t_i).scalar_tensor_tensor(
                        acc[n][t_i][:], ht[:], w2col(c),
                        acc[n][t_i][:], op0=ALU.mult, op1=ALU.add)
                if c >= CC - NT:
                    # chain for tile t_i is complete -> emit the centered
                    # fp8 activation now so fc3's matmuls can start before
                    # the last chain
                    jj, e = t_i // 2, t_i % 2
                    if x2_sb[n][jj] is None:
                        x2_sb[n][jj] = x2pool.tile(
                            [128, 2, NBLK], fp8, tag=f"x2_{n}_{jj}",
                            name=f"x2sb_{n}_{jj}")
                    nc.scalar.activation(x2_sb[n][jj][:, e:e + 1, :],
                                         acc[n][t_i][:], AF.Tanh,
                                         bias=b2col(t_i), scale=0.5)

            def fc1_block_jouter(n, split_j01=False):
                # j-outer over the first column-block: each j-pair of x/W1
                # arrives as its own 128KB DMA, so matmuls start as soon as
                # the first pair lands; the whole head (x_0..x_3) is
                # DMA-bandwidth-limited (~220GB/s effective), so all of
                # group0's phases consume j-pair slices in arrival order.
                # For n=0, j0 and j1 run as single-subtile normal-mode fp8
                # matmuls (same N cycles/col as a DR pair) so the very
                # first matmul needs only 64KB of x + 32KB of W1.
                phs = [psum_h_pool.tile([128, NBLK], f32, tag="psum_h",
                                        name=f"ph0_{n}_{c}")
                       for c in range(CBLK)]
                if split_j01:
                    for j in (0, 1):
                        for c in range(CBLK):
                            nc.tensor.matmul(
                                phs[c][:],
                                lhsT=w1_sb[:, j:j + 1,
                                           128 * c:128 * (c + 1)],
                                rhs=x_sb[n][:, j:j + 1, :],
                                start=(j == 0),
                                stop=False,
                                skip_group_check=True,
                            )
                    j_start = 1
                else:
                    j_start = 0
                for j in range(j_start, KP):
                    for c in range(CBLK):
                        nc.tensor.matmul(
                            phs[c][:],
                            lhsT=w1_sb[:, 2 * j:2 * j + 2,
                                       128 * c:128 * (c + 1)],
                            rhs=x_sb[n][:, 2 * j:2 * j + 2, :],
                            start=(j == 0 and not split_j01),
                            stop=(j == KP - 1),
                            perf_mode=DR,
                            skip_group_check=True,
                        )
                for c in range(CBLK):
                    ht = hpool.tile([128, NBLK], bf16, tag="ht",
                                    name=f"ht_{n}0_{c}")
                    nc.scalar.activation(ht[:], phs[c][:], AF.Sigmoid,
                                         bias=b1col(c))
                    fc2_step(c, n, ht)

            def fc1_block(cs, n, split_last=False):
                for c in cs:
                    split = split_last and c == cs[-1]
                    ph = psum_h_pool.tile([128, NBLK], f32, tag="psum_h",
                                          name=f"ph_{n}_{c}")
                    if not split:
                        for j in range(KP):
                            nc.tensor.matmul(
                                ph[:],
                                lhsT=w1_sb[:, 2 * j:2 * j + 2,
                                           128 * c:128 * (c + 1)],
                                rhs=x_sb[n][:, 2 * j:2 * j + 2, :],
                                start=(j == 0),
                                stop=(j == KP - 1),
                                perf_mode=DR,
                            )
                        ht = hpool.tile([128, NBLK], bf16, tag="ht",
                                        name=f"ht_{n}_{c}")
                        nc.scalar.activation(ht[:], ph[:], AF.Sigmoid,
                                             bias=b1col(c))
                        fc2_step(c, n, ht)
                        continue
                    # final chunk of the final row-block: half-width (256)
                    # pipeline so the serial ht->acc->x2 chain at the very
                    # end runs on half tiles, overlapped across halves
                    t_i = c % NT
                    jj, e = t_i // 2, t_i % 2
                    xt = x2_sb[n][jj]  # allocated at the e=0 completion
                    ht = hpool.tile([128, NBLK], bf16, tag="ht",
                                    name=f"ht_{n}_{c}")
                    H = NBLK // 2
                    for h in range(2):
                        s = slice(h * H, (h + 1) * H)
                        for j in range(KP):
                            nc.tensor.matmul(
                                ph[:, s],
                                lhsT=w1_sb[:, 2 * j:2 * j + 2,
                                           128 * c:128 * (c + 1)],
                                rhs=x_sb[n][:, 2 * j:2 * j + 2, s],
                                start=(j == 0),
                                stop=(j == KP - 1),
                                perf_mode=DR,
                                skip_group_check=True,
                            )
                        nc.scalar.activation(ht[:, s], ph[:, s], AF.Sigmoid,
                                             bias=b1col(c))
                        fc2_eng(t_i).scalar_tensor_tensor(
                            acc[n][t_i][:, s], ht[:, s], w2col(c),
                            acc[n][t_i][:, s], op0=ALU.mult, op1=ALU.add)
                        nc.scalar.activation(xt[:, e:e + 1, s],
                                             acc[n][t_i][:, s],
                                             AF.Tanh, bias=b2col(t_i),
                                             scale=0.5)

            def fc3_block(n, last=False):
                # fp8 DoubleRow fc3: 2 passes of K=256 (x2 pair tiles)
                # instead of 4 bf16 K=128 passes — half the matmul cycles.
                # jj-outer issue order: jj=0 (chains t0/t1, done early) full
                # width first; jj=1 (t2/t3, the late chains) half width so
                # the last chain's matmuls pipeline with the bias/DMA
                # epilogue.
                po = [psum_o_pool.tile([128, NBLK], f32, tag="psum_h",
                                       name=f"po_{n}_{d}")
                      for d in range(ND)]
                for d in range(ND):
                    nc.tensor.matmul(
                        po[d][:],
                        lhsT=w3_sb[:, 0, :, 128 * d:128 * (d + 1)],
                        rhs=x2_sb[n][0][:],
                        start=True,
                        stop=False,
                        perf_mode=DR,
                        skip_group_check=True,
                    )
                H = NBLK // 2
                for h in range(2):
                    s = slice(h * H, (h + 1) * H)
                    for d in range(ND):
                        nc.tensor.matmul(
                            po[d][:, s],
                            lhsT=w3_sb[:, 1, :, 128 * d:128 * (d + 1)],
                            rhs=x2_sb[n][1][:, :, s],
                            start=False,
                            stop=(h == 1),
                            perf_mode=DR,
                            skip_group_check=True,
                        )
                if last:
                    # halves pipelined with the t3 half-matmuls: po[d]'s
                    # cols 0:H are final right after the (h0, d) matmul, so
                    # the h0 bias+DMA overlap the h1 matmuls.  d0 rides
                    # DVE+sync, d1 rides ACT(Identity+bias)+scalar.  (A
                    # gpsimd DMA here costs a ~2.1us SWDGE ring DRAIN.)
                    ots = [opool.tile([128, NBLK], bf16, tag="ot",
                                      name=f"ot_{n}_{d}")
                           for d in range(ND)]
                    for h in range(2):
                        s = slice(h * H, (h + 1) * H)
                        for d in range(ND):
                            if d == 0:
                                nc.vector.tensor_scalar(
                                    ots[d][:, s], po[d][:, s], 0.25,
                                    b3col(d), op0=ALU.mult, op1=ALU.add)
                            else:
                                nc.scalar.activation(
                                    ots[d][:, s], po[d][:, s], AF.Identity,
                                    bias=b3col(d), scale=0.25)
                            dma_eng = nc.sync if d == 0 else nc.scalar
                            dma_eng.dma_start(
                                out.ap()[128 * d:128 * (d + 1),
                                         n * NBLK + h * H:
                                         n * NBLK + (h + 1) * H],
                                ots[d][:, s])
                    return
                for d in range(ND):
                    ot = opool.tile([128, NBLK], bf16, tag="ot",
                                    name=f"ot_{n}_{d}")
                    # d0 bias on DVE, d1 on ACT (Identity+bias): all fc3
                    # blocks run in the endgame where DVE is saturated by
                    # the fc2 STT chains while ACT has slack; both DMAs on
                    # sync (a scalar-queue DMA would block the remaining
                    # sigmoids)
                    if d == 0:
                        nc.vector.tensor_scalar(ot[:], po[d][:], 0.25,
                                                b3col(d), op0=ALU.mult,
                                                op1=ALU.add)
                    else:
                        nc.scalar.activation(ot[:], po[d][:], AF.Identity,
                                             bias=b3col(d), scale=0.25)
                    nc.sync.dma_start(
                        out.ap()[128 * d:128 * (d + 1),
                                 n * NBLK:(n + 1) * NBLK], ot[:])

            # --- fc1 + fc2: column-blocks of CBLK c-chunks, n-outer inside
            # so each x tile's DMA arrival unlocks a block of work; each
            # psum tile's 4 matmuls are consecutive (liveness ~1 bank).
            # In the last block, each row-block's fc3 is interleaved one
            # n-phase behind its fc1 so the x2 sigmoid chains are covered
            # by other matmul work. ---
            NORD = (0, 1, 3, 2)  # x DMA arrival order: x1 sync#4 (~13.5us),
            # x3 sync#5 (~15.5us), x2 gpsimd#4 (~16us)
            # Chain updates are commutative.  Group the last 16 chunks so
            # chains t0/t1/t2 each complete inside a 5-chunk group (their
            # 4-serial STT chain and x2 sigmoid land early, inside a
            # 4.3us phase with ACT slack), while chain t3's first three
            # chunks ride along and ONLY c31 sits in the final single-chunk
            # group: the end-of-kernel chain is then one sigmoid + one STT
            # + one sigmoid instead of a 4-serial STT cluster.
            groups = [list(range(4 * g, 4 * g + 4)) for g in range(4)] + [
                [16, 20, 24, 28, 19],
                [17, 21, 25, 29, 23],
                [18, 22, 26, 30],
                [27, 31],
            ]
            # final group runs n=1 LAST (order 0,2,3,1); fc3(0) is emitted
            # BEFORE the final n=1 phase (its inputs are ready — block 0's
            # chains complete during the n=2/n=3 phases) so the shortened
            # fp8-DR fc3 stream [fc3(2), fc3(3), fc3(1)] still covers block
            # 1's final ht->acc->x2 chain, and fc3(1) is deferred to the
            # very end.
            LAST_ORD = (0, 2, 3, 1)
            for gi, chunks in enumerate(groups):
                last_b = gi == len(groups) - 1
                for n in (LAST_ORD if last_b else NORD):
                    if last_b and n == LAST_ORD[-1]:
                        fc3_block(0)
                    if gi == 0:
                        fc1_block_jouter(n)
                    else:
                        fc1_block(chunks, n,
                                  split_last=(last_b and n == LAST_ORD[-1]))
            for n in LAST_ORD[1:]:
                fc3_block(n, last=(n == LAST_ORD[-1]))

    nc.compile()
    return nc


def get_nc():
    if "nc" not in _compiled:
        _compiled["nc"] = _build_nc()
    return _compiled["nc"]


def make_in_maps(x, W1, b1, W2, b2, W3, b3):
    x = np.asarray(x, dtype=np.float32)
    W1 = np.asarray(W1, dtype=np.float32)
    b1 = np.asarray(b1, dtype=np.float32)
    W2 = np.asarray(W2, dtype=np.float32)
    b2 = np.asarray(b2, dtype=np.float32)
    W3 = np.asarray(W3, dtype=np.float32)
    b3 = np.asarray(b3, dtype=np.float32)

    # s-major permutation of H1: new index p = s*H2 + g  (old h1 = g*GS + s)
    p = np.arange(H1)
    perm = (p % H2) * GS + (p // H2)
    W1p = W1[perm, :]
    b1p = b1[perm]

    # fp8 fc1 operands in DoubleRow layout [128, KC, *]:
    # element (p, j, m) holds contraction index k = 128*j + p
    w1t = W1p.T.astype(FP8)  # [D_IN, H1]
    w1q_h = np.ascontiguousarray(
        w1t.reshape(KC, 128, H1).transpose(1, 0, 2))
    xt = x.T.astype(FP8)  # [D_IN, B]
    xq_h = np.ascontiguousarray(
        xt.reshape(KC, 128, B).transpose(1, 0, 2))

    b1c_h = b1p.reshape(CC, 128).T
    # chunk c: s = c//NT, tile t = c%NT, partition k <-> group 128*t + k
    w2c_h = np.empty((128, CC), dtype=np.float32)
    for c in range(CC):
        w2c_h[:, c] = W2[128 * (c % NT):128 * (c % NT) + 128, c // NT]
    # fc2 output is emitted as tanh((z+b2)/2) = sigmoid(z+b2) - 0.5 via
    # ACT(Tanh, scale=0.5, bias=b2/2); fc3 runs on W3' = 2*W3 in fp8
    # DoubleRow and the kernel applies out = 0.25*psum + b3' with
    # b3' = b3 + 0.5*W3.sum(1)
    b2c_h = (0.5 * b2).reshape(NT, 128).T
    b3p = b3 + 0.5 * W3.sum(axis=1)
    b3c_h = b3p.reshape(ND, 128).T
    cst_h = np.ascontiguousarray(
        np.concatenate([b1c_h, w2c_h, b2c_h, b3c_h], axis=1),
        dtype=np.float32)  # [128, 2*CC + NT + ND]
    w3t = (2.0 * W3).T.astype(FP8)  # [H2, D_OUT]
    w3q_h = np.ascontiguousarray(
        w3t.reshape(2, 2, 128, D_OUT).transpose(2, 0, 1, 3))

    in_maps = []
    for i in range(N_CORES):
        in_maps.append({
            "xq": np.ascontiguousarray(
                xq_h[:, :, i * B_SHARD:(i + 1) * B_SHARD]),
            "w1q": w1q_h,
            "cst": cst_h,
            "w3q": w3q_h,
        })
    return in_maps


def kernel(x, W1, b1, W2, b2, W3, b3):
    import os
    from concourse.bass_utils import run_bass_kernel_spmd

    nc = get_nc()
    in_maps = make_in_maps(x, W1, b1, W2, b2, W3, b3)
    # force tracing off for this call: the agent image lacks the axon NTFF
    # hook module, so a stray BASS_TRACE=1 would crash the run
    prev = os.environ.get("BASS_NEVER_TRACE")
    os.environ["BASS_NEVER_TRACE"] = "1"
    try:
        res = run_bass_kernel_spmd(nc, in_maps, core_ids=list(range(N_CORES)))
    finally:
        if prev is None:
            os.environ.pop("BASS_NEVER_TRACE", None)
        else:
            os.environ["BASS_NEVER_TRACE"] = prev
    outT = np.concatenate([res.results[i]["out"].astype(np.float32)
                           for i in range(N_CORES)], axis=1)  # [D_OUT, B]
    return np.ascontiguousarray(outT.T)



# revision 53
# speedup vs baseline: 1.0060x; 1.0060x over previous
"""Trainium2 Bass kernel for AdaptiveNet MLP (fc1+sigmoid, grouped fc2+sigmoid, fc3).

Sharding: pure data-parallel over batch across 8 NeuronCores (no collectives).
Each core computes its 2048-row shard through all three layers.

fc1 (95% of FLOPs) runs in fp8-e4m3 with DoubleRow perf mode (two fp8 weights
per PE cell -> K=256 per matmul, halving the matmul count); the sigmoid damps
the quantization error so the final rel-err stays ~3e-3 (gate is 2e-2).

Layout trick: H1 is permuted s-major on the host (h1' = s*512 + g, where the
original h1 = g*8 + s).  fc1 then produces hT' tiles [128 h1' partitions x 512
rows]; the grouped fc2 contraction over s becomes 8 fused multiply-accumulate
ops on the vector engine with per-partition scalars (W2 columns), and fc3 is a
plain bf16 matmul over the 512 groups.  Biases are per-partition [128,1]
columns fused into ScalarE sigmoids / a VectorE add.

The PE roofline for this shape is ~110.6us of fc1 matmuls (N=512 columns at
1 col/cycle @2.4GHz, fp8-DR K=256/pass — measured: every N=512 matmul costs
216ns regardless of dtype/perf-mode) + ~7us fc3, behind a fixed ~7.1us NEFF
preamble.  Overlap work to sit on that roofline:

- PE warmup: 14 throwaway DR matmuls on a Pool-memset scratch tile keep the
  PE busy from ~7.7us until the first real slices land (~11us); a PE idle
  before the DVFS ramp completes costs ~1-2us of 1.2GHz matmuls.
- DMA head: the first ~22us is bandwidth-limited (~220GB/s effective).  All
  of group0's phases run j-outer and x_0..x_3 arrive as per-j-pair 128KB
  slices in exact consumption order.  Lane rules learned from traces: the
  sync HWDGE ring is the only fast multi-DMA queue; the scalar ring delivers
  its 2nd+ DMA ~4us late (gets only x_0 j01); gpsimd SWDGE takes max 4 DMAs
  (a 5th DRAIN-blocks its ring ~10us).
- fc2 chain scheduling: the last 16 chunks are grouped [16,20,24,28,19],
  [17,21,25,29,23], [18,22,26,30,27], [31] so chains t0..t2 complete inside
  4.3us phases with ACT slack, and the end-of-kernel chain for t3 is a
  single sigmoid+STT+sigmoid.
- PSUM: fc1 chunks cycle a 5-bank pool, fc3 po tiles a separate 3-bank pool
  (sharing one pool put fc3(1)'s banks behind the final sigmoid chain).
- Tail: fc3 runs t-outer with the final chunk half-width; fc3(1) is
  deferred last so its t0..t2 matmuls cover the final sigmoid chain; the
  last block's bias+output run as halves pipelined with the t3 matmuls,
  d0 on DVE+sync and d1 on ACT(Identity+bias)+scalar.
"""

import sys

for _p in ("/opt/trn_rl_repo",):
    if _p not in sys.path:
        sys.path.append(_p)

import numpy as np
import ml_dtypes

BF16 = ml_dtypes.bfloat16
FP8 = ml_dtypes.float8_e4m3  # == mybir.dt.float8e4

D_IN, H1, H2, D_OUT = 1024, 4096, 512, 256
GS = H1 // H2  # 8
B = 16384
N_CORES = 8
B_SHARD = B // N_CORES  # 2048
NBLK = 512  # rows per block (one PSUM bank of fp32)
NB = B_SHARD // NBLK  # 4
KC = D_IN // 128  # 8 contraction subtiles for fc1
KP = KC // 2  # 4 DoubleRow pairs
CC = H1 // 128  # 32 h1' chunks
NT = H2 // 128  # 4 x2T tiles
ND = D_OUT // 128  # 2 output chunks

_compiled = {}


def _build_nc():
    from concourse import bacc, tile, mybir

    f32 = mybir.dt.float32
    bf16 = mybir.dt.bfloat16
    fp8 = mybir.dt.float8e4
    AF = mybir.ActivationFunctionType
    ALU = mybir.AluOpType
    DR = mybir.MatmulPerfMode.DoubleRow

    nc = bacc.Bacc("TRN2", target_bir_lowering=False, debug=False,
                   num_devices=N_CORES)

    xq = nc.dram_tensor("xq", [128, KC, B_SHARD], fp8, kind="ExternalInput")
    w1q = nc.dram_tensor("w1q", [128, KC, H1], fp8, kind="ExternalInput")
    # all [128, *] f32 constants packed on the free axis:
    # b1 (CC) | w2 (CC) | b2 (NT) | b3 (ND)
    cst = nc.dram_tensor("cst", [128, 2 * CC + NT + ND], f32,
                         kind="ExternalInput")
    # W3' = 2*W3 in fp8, DoubleRow pair-interleaved: [p, jj, e, m] holds
    # w3t'[128*(2jj+e)+p, m]
    w3q = nc.dram_tensor("w3q", [128, 2, 2, D_OUT], fp8, kind="ExternalInput")
    out = nc.dram_tensor("out", [D_OUT, B_SHARD], bf16, kind="ExternalOutput")

    with tile.TileContext(nc) as tc:
        with (
            tc.tile_pool(name="wpool", bufs=1) as wpool,
            tc.tile_pool(name="xpool", bufs=1) as xpool,
            tc.tile_pool(name="hpool", bufs=8) as hpool,
            tc.tile_pool(name="accpool", bufs=1) as accpool,
            tc.tile_pool(name="x2pool", bufs=1) as x2pool,
            tc.tile_pool(name="opool", bufs=4) as opool,
            tc.tile_pool(name="psum_h", bufs=5, space="PSUM") as psum_h_pool,
            tc.tile_pool(name="psum_o", bufs=3, space="PSUM") as psum_o_pool,
        ):
            # fc3 po tiles get their own 3-bank pool: sharing all 8 banks
            # with fc1 made fc3(1)'s po reuse the bank of block1's final
            # ph chunk, which frees only after the last sigmoid chain ->
            # ~1.4us PE stall right before the last matmuls
            w1_sb = wpool.tile([128, KC, H1], fp8, tag="w1")
            x_sb = [None] * NB
            for n in range(NB):
                x_sb[n] = xpool.tile([128, KC, NBLK], fp8,
                                     tag=f"x_{n}", name=f"xsb_{n}")
            # Four issuing queues (sync/scalar HWDGE ~0.6us first-byte,
            # vector/gpsimd SWDGE ~1us), FIFO within each.  The NEFF preamble
            # (engine sem barrier + instruction-stream loads) runs to ~7.1us,
            # so the first dma_start issues ~7.2us and the first 32-128KB
            # slices land ~9.5-10us.  Slice the head so the very first
            # matmuls (block0, j-outer) unblock on 32KB granularity.
            CBLK = 4
            WBLK = 8

            cst_sb = wpool.tile([128, 2 * CC + NT + ND], f32, tag="cst")
            w3_sb = wpool.tile([128, 2, 2, D_OUT], fp8, tag="w3q")

            # --- PE warmup: keep the PE busy through the input-DMA head so
            # the HAM clock gate flips to 8/8 before real matmuls arrive and
            # the cold 1.2GHz window is spent on throwaway work.  The scratch
            # tile is memset on the Pool queue (first op, ~0.1us) so warmup
            # matmuls start ~7.4us; 10 N=256 DR matmuls cover until ~9.8us
            # when the first real slices land. ---
            wm = wpool.tile([128, 2, 256], fp8, tag="wm")
            nc.gpsimd.memset(wm[:], 0)
            wm_ps = psum_h_pool.tile([128, 256], f32, tag="psum_h",
                                     name="wm_ps")
            for _wi in range(14):
                nc.tensor.matmul(wm_ps[:], lhsT=wm[:, :, 0:128], rhs=wm[:],
                                 start=True, stop=True,
                                 perf_mode=DR)

            def b1col(c):
                return cst_sb[:, c:c + 1]

            def w2col(c):
                return cst_sb[:, CC + c:CC + c + 1]

            def b2col(t):
                return cst_sb[:, 2 * CC + t:2 * CC + t + 1]

            def b3col(d):
                return cst_sb[:, 2 * CC + NT + d:2 * CC + NT + d + 1]

            # one DMA per W1 column-block covering all subtile pairs; the
            # first block split in half so the very first matmuls unblock
            # sooner
            def wblock(cb0, cb1, eng):
                c0, c1 = cb0 * 128, cb1 * 128
                eng.dma_start(w1_sb[:, :, c0:c1], w1q.ap()[:, :, c0:c1])

            def xdma(n, eng):
                eng.dma_start(x_sb[n][:],
                              xq.ap()[:, :, n * NBLK:(n + 1) * NBLK])

            # Lane layout tuned so block0's j-outer stream is gapless from
            # first matmul (a PE idle before the clock ramp completes resets
            # it to 1.2GHz for ~4us).  Per-j-pair arrivals interleave across
            # sync/scalar (HWDGE) and gpsimd (SWDGE, max 4 DMAs or its ring
            # DRAIN-blocks for ~10us).
            # sync (fast HWDGE ring): all block0 W1 j-pair slices in
            # consumption order, then x_1/x_3/x_2, the big W1 blocks, w3.
            # The scalar ring delivers its 2nd+ DMA ~4us late, so it gets
            # only x_0 j01; gpsimd (SWDGE, max 4 DMAs) takes the other x_0
            # j-pair slices + the small consts.
            nc.sync.dma_start(w1_sb[:, 0:2, 0:CBLK * 128],
                              w1q.ap()[:, 0:2, 0:CBLK * 128])
            nc.sync.dma_start(w1_sb[:, 2:4, 0:CBLK * 128],
                              w1q.ap()[:, 2:4, 0:CBLK * 128])
            nc.sync.dma_start(w1_sb[:, 4:6, 0:CBLK * 128],
                              w1q.ap()[:, 4:6, 0:CBLK * 128])
            nc.sync.dma_start(w1_sb[:, 6:8, 0:CBLK * 128],
                              w1q.ap()[:, 6:8, 0:CBLK * 128])
            # x_1/x_3/x_2 per-j-pair slices in exact consumption order of
            # the j-outer group0 phases (NORD = 0,1,3,2)
            for nn in (1, 3, 2):
                for j in range(KP):
                    nc.sync.dma_start(
                        x_sb[nn][:, 2 * j:2 * j + 2, :],
                        xq.ap()[:, 2 * j:2 * j + 2,
                                nn * NBLK:(nn + 1) * NBLK])
            for cb in range(CBLK, CC, WBLK):
                wblock(cb, min(cb + WBLK, CC), nc.sync)
            nc.sync.dma_start(w3_sb[:], w3q.ap()[:])
            # scalar: x_0 j01 only (gates the first real matmul; this
            # queue's 2nd DMA arrives ~4us late, so it gets just one)
            nc.scalar.dma_start(x_sb[0][:, 0:2, :], xq.ap()[:, 0:2, 0:NBLK])
            # gpsimd: x_0 j23/j45/j67 + consts (exactly 4 DMAs; a 5th
            # DRAIN-blocks its SWDGE ring for ~10us)
            nc.gpsimd.dma_start(x_sb[0][:, 2:4, :], xq.ap()[:, 2:4, 0:NBLK])
            nc.gpsimd.dma_start(x_sb[0][:, 4:6, :], xq.ap()[:, 4:6, 0:NBLK])
            nc.gpsimd.dma_start(x_sb[0][:, 6:8, :], xq.ap()[:, 6:8, 0:NBLK])
            nc.gpsimd.dma_start(cst_sb[:], cst.ap()[:])

            # fc2 accumulators, one per (row-block, x2 tile)
            acc = [[None] * NT for _ in range(NB)]

            # x2 activations in fp8, paired per DoubleRow k-pair jj:
            # tile [128, 2, NBLK] holds tanh((z+b2)/2) for t=2jj (e=0) and
            # t=2jj+1 (e=1).  tanh((z+b2)/2) = sigmoid(z+b2) - 0.5 exactly;
            # the 0.5 shift and 2x tanh scale fold into b3'/W3' on the host
            # (b3' = b3 + 0.5*W3.sum(1), W3' = 2*W3, out = 0.25*psum + b3').
            # Centering halves the fp8 quantization error; measured total
            # rel-err 7.7e-3 vs the 2e-2 gate.
            x2_sb = [[None] * 2 for _ in range(NB)]

            # fc2 accumulate chains all on DVE: Pool (= the GPSIMD Q7
            # complex on TRN2) has no TensorScalarPtr opcode, is ~2x slower
            # for elementwise, and shares its SBUF port with DVE anyway
            def fc2_eng(t_i):
                return nc.vector

            def fc2_step(c, n, ht):
                t_i = c % NT
                if c < NT:
                    acc[n][t_i] = accpool.tile([128, NBLK], bf16,
                                               tag=f"acc_{n}_{t_i}",
                                               name=f"acc_{n}_{t_i}")
                    fc2_eng(t_i).tensor_scalar_mul(acc[n][t_i][:], ht[:],
                                                   w2col(c))
                else:
                    fc2_eng(t_i).scalar_tensor_tensor(
                        acc[n][t_i][:], ht[:], w2col(c),
                        acc[n][t_i][:], op0=ALU.mult, op1=ALU.add)
                if c >= CC - NT:
                    # chain for tile t_i is complete -> emit the centered
                    # fp8 activation now so fc3's matmuls can start before
                    # the last chain
                    jj, e = t_i // 2, t_i % 2
                    if x2_sb[n][jj] is None:
                        x2_sb[n][jj] = x2pool.tile(
                            [128, 2, NBLK], fp8, tag=f"x2_{n}_{jj}",
                            name=f"x2sb_{n}_{jj}")
                    nc.scalar.activation(x2_sb[n][jj][:, e:e + 1, :],
                                         acc[n][t_i][:], AF.Tanh,
                                         bias=b2col(t_i), scale=0.5)

            def fc1_block_jouter(n, split_j01=False):
                # j-outer over the first column-block: each j-pair of x/W1
                # arrives as its own 128KB DMA, so matmuls start as soon as
                # the first pair lands; the whole head (x_0..x_3) is
                # DMA-bandwidth-limited (~220GB/s effective), so all of
                # group0's phases consume j-pair slices in arrival order.
                # For n=0, j0 and j1 run as single-subtile normal-mode fp8
                # matmuls (same N cycles/col as a DR pair) so the very
                # first matmul needs only 64KB of x + 32KB of W1.
                phs = [psum_h_pool.tile([128, NBLK], f32, tag="psum_h",
                                        name=f"ph0_{n}_{c}")
                       for c in range(CBLK)]
                if split_j01:
                    for j in (0, 1):
                        for c in range(CBLK):
                            nc.tensor.matmul(
                                phs[c][:],
                                lhsT=w1_sb[:, j:j + 1,
                                           128 * c:128 * (c + 1)],
                                rhs=x_sb[n][:, j:j + 1, :],
                                start=(j == 0),
                                stop=False,
                                skip_group_check=True,
                            )
                    j_start = 1
                else:
                    j_start = 0
                for j in range(j_start, KP):
                    for c in range(CBLK):
                        nc.tensor.matmul(
                            phs[c][:],
                            lhsT=w1_sb[:, 2 * j:2 * j + 2,
                                       128 * c:128 * (c + 1)],
                            rhs=x_sb[n][:, 2 * j:2 * j + 2, :],
                            start=(j == 0 and not split_j01),
                            stop=(j == KP - 1),
                            perf_mode=DR,
                            skip_group_check=True,
                        )
                for c in range(CBLK):
                    ht = hpool.tile([128, NBLK], bf16, tag="ht",
                                    name=f"ht_{n}0_{c}")
                    nc.scalar.activation(ht[:], phs[c][:], AF.Sigmoid,
                                         bias=b1col(c))
                    fc2_step(c, n, ht)

            def fc1_block(cs, n, split_last=False):
                for c in cs:
                    split = split_last and c == cs[-1]
                    ph = psum_h_pool.tile([128, NBLK], f32, tag="psum_h",
                                          name=f"ph_{n}_{c}")
                    if not split:
                        for j in range(KP):
                            nc.tensor.matmul(
                                ph[:],
                                lhsT=w1_sb[:, 2 * j:2 * j + 2,
                                           128 * c:128 * (c + 1)],
                                rhs=x_sb[n][:, 2 * j:2 * j + 2, :],
                                start=(j == 0),
                                stop=(j == KP - 1),
                                perf_mode=DR,
                            )
                        ht = hpool.tile([128, NBLK], bf16, tag="ht",
                                        name=f"ht_{n}_{c}")
                        nc.scalar.activation(ht[:], ph[:], AF.Sigmoid,
                                             bias=b1col(c))
                        fc2_step(c, n, ht)
                        continue
                    # final chunk of the final row-block: half-width (256)
                    # pipeline so the serial ht->acc->x2 chain at the very
                    # end runs on half tiles, overlapped across halves
                    t_i = c % NT
                    jj, e = t_i // 2, t_i % 2
                    xt = x2_sb[n][jj]  # allocated at the e=0 completion
                    ht = hpool.tile([128, NBLK], bf16, tag="ht",
                                    name=f"ht_{n}_{c}")
                    H = NBLK // 2
                    for h in range(2):
                        s = slice(h * H, (h + 1) * H)
                        for j in range(KP):
                            nc.tensor.matmul(
                                ph[:, s],
                                lhsT=w1_sb[:, 2 * j:2 * j + 2,
                                           128 * c:128 * (c + 1)],
                                rhs=x_sb[n][:, 2 * j:2 * j + 2, s],
                                start=(j == 0),
                                stop=(j == KP - 1),
                                perf_mode=DR,
                                skip_group_check=True,
                            )
                        nc.scalar.activation(ht[:, s], ph[:, s], AF.Sigmoid,
                                             bias=b1col(c))
                        fc2_eng(t_i).scalar_tensor_tensor(
                            acc[n][t_i][:, s], ht[:, s], w2col(c),
                            acc[n][t_i][:, s], op0=ALU.mult, op1=ALU.add)
                        nc.scalar.activation(xt[:, e:e + 1, s],
                                             acc[n][t_i][:, s],
                                             AF.Tanh, bias=b2col(t_i),
                                             scale=0.5)

            def fc3_block(n, last=False):
                # fp8 DoubleRow fc3: 2 passes of K=256 (x2 pair tiles)
                # instead of 4 bf16 K=128 passes — half the matmul cycles.
                # jj-outer issue order: jj=0 (chains t0/t1, done early) full
                # width first; jj=1 (t2/t3, the late chains) half width so
                # the last chain's matmuls pipeline with the bias/DMA
                # epilogue.
                po = [psum_o_pool.tile([128, NBLK], f32, tag="psum_h",
                                       name=f"po_{n}_{d}")
                      for d in range(ND)]
                for d in range(ND):
                    nc.tensor.matmul(
                        po[d][:],
                        lhsT=w3_sb[:, 0, :, 128 * d:128 * (d + 1)],
                        rhs=x2_sb[n][0][:],
                        start=True,
                        stop=False,
                        perf_mode=DR,
                        skip_group_check=True,
                    )
                H = NBLK // 2
                for h in range(2):
                    s = slice(h * H, (h + 1) * H)
                    for d in range(ND):
                        nc.tensor.matmul(
                            po[d][:, s],
                            lhsT=w3_sb[:, 1, :, 128 * d:128 * (d + 1)],
                            rhs=x2_sb[n][1][:, :, s],
                            start=False,
                            stop=(h == 1),
                            perf_mode=DR,
                            skip_group_check=True,
                        )
                if last:
                    # halves pipelined with the t3 half-matmuls: po[d]'s
                    # cols 0:H are final right after the (h0, d) matmul, so
                    # the h0 bias+DMA overlap the h1 matmuls.  d0 rides
                    # DVE+sync, d1 rides ACT(Identity+bias)+scalar.  (A
                    # gpsimd DMA here costs a ~2.1us SWDGE ring DRAIN.)
                    ots = [opool.tile([128, NBLK], bf16, tag="ot",
                                      name=f"ot_{n}_{d}")
                           for d in range(ND)]
                    for h in range(2):
                        s = slice(h * H, (h + 1) * H)
                        for d in range(ND):
                            if d == 0:
                                nc.vector.tensor_scalar(
                                    ots[d][:, s], po[d][:, s], 0.25,
                                    b3col(d), op0=ALU.mult, op1=ALU.add)
                            else:
                                nc.scalar.activation(
                                    ots[d][:, s], po[d][:, s], AF.Identity,
                                    bias=b3col(d), scale=0.25)
                            dma_eng = nc.sync if d == 0 else nc.scalar
                            dma_eng.dma_start(
                                out.ap()[128 * d:128 * (d + 1),
                                         n * NBLK + h * H:
                                         n * NBLK + (h + 1) * H],
                                ots[d][:, s])
                    return
                for d in range(ND):
                    ot = opool.tile([128, NBLK], bf16, tag="ot",
                                    name=f"ot_{n}_{d}")
                    # d0 bias on DVE, d1 on ACT (Identity+bias): all fc3
                    # blocks run in the endgame where DVE is saturated by
                    # the fc2 STT chains while ACT has slack; both DMAs on
                    # sync (a scalar-queue DMA would block the remaining
                    # sigmoids)
                    if d == 0:
                        nc.vector.tensor_scalar(ot[:], po[d][:], 0.25,
                                                b3col(d), op0=ALU.mult,
                                                op1=ALU.add)
                    else:
                        nc.scalar.activation(ot[:], po[d][:], AF.Identity,
                                             bias=b3col(d), scale=0.25)
                    nc.sync.dma_start(
                        out.ap()[128 * d:128 * (d + 1),
                                 n * NBLK:(n + 1) * NBLK], ot[:])

            # --- fc1 + fc2: column-blocks of CBLK c-chunks, n-outer inside
            # so each x tile's DMA arrival unlocks a block of work; each
            # psum tile's 4 matmuls are consecutive (liveness ~1 bank).
            # In the last block, each row-block's fc3 is interleaved one
            # n-phase behind its fc1 so the x2 sigmoid chains are covered
            # by other matmul work. ---
            NORD = (0, 1, 3, 2)  # x DMA arrival order: x1 sync#4 (~13.5us),
            # x3 sync#5 (~15.5us), x2 gpsimd#4 (~16us)
            # Chain updates are commutative.  Group the last 16 chunks so
            # chains t0/t1/t2 each complete inside a 5-chunk group (their
            # 4-serial STT chain and x2 sigmoid land early, inside a
            # 4.3us phase with ACT slack), while chain t3's first three
            # chunks ride along and ONLY c31 sits in the final single-chunk
            # group: the end-of-kernel chain is then one sigmoid + one STT
            # + one sigmoid instead of a 4-serial STT cluster.
            groups = [list(range(4 * g, 4 * g + 4)) for g in range(4)] + [
                [16, 20, 24, 28, 19],
                [17, 21, 25, 29, 23],
                [18, 22, 26, 30, 27],
                [31],
            ]
            # final group runs n=1 LAST (order 0,2,3,1); fc3(0) is emitted
            # BEFORE the final n=1 phase (its inputs are ready — block 0's
            # chains complete during the n=2/n=3 phases) so the shortened
            # fp8-DR fc3 stream [fc3(2), fc3(3), fc3(1)] still covers block
            # 1's final ht->acc->x2 chain, and fc3(1) is deferred to the
            # very end.
            LAST_ORD = (0, 2, 3, 1)
            for gi, chunks in enumerate(groups):
                last_b = gi == len(groups) - 1
                for n in (LAST_ORD if last_b else NORD):
                    if last_b and n == LAST_ORD[-1]:
                        fc3_block(0)
                    if gi == 0:
                        fc1_block_jouter(n)
                    else:
                        # in the final single-chunk group, EVERY n-phase
                        # runs the half-width split pipeline: fc3(n)'s jj1
                        # matmuls gate on the c31 chain, and halving it
                        # releases them ~0.8us sooner per phase
                        fc1_block(chunks, n, split_last=last_b)
            for n in LAST_ORD[1:]:
                fc3_block(n, last=(n == LAST_ORD[-1]))

    nc.compile()
    return nc


def get_nc():
    if "nc" not in _compiled:
        _compiled["nc"] = _build_nc()
    return _compiled["nc"]


def make_in_maps(x, W1, b1, W2, b2, W3, b3):
    x = np.asarray(x, dtype=np.float32)
    W1 = np.asarray(W1, dtype=np.float32)
    b1 = np.asarray(b1, dtype=np.float32)
    W2 = np.asarray(W2, dtype=np.float32)
    b2 = np.asarray(b2, dtype=np.float32)
    W3 = np.asarray(W3, dtype=np.float32)
    b3 = np.asarray(b3, dtype=np.float32)

    # s-major permutation of H1: new index p = s*H2 + g  (old h1 = g*GS + s)
    p = np.arange(H1)
    perm = (p % H2) * GS + (p // H2)
    W1p = W1[perm, :]
    b1p = b1[perm]

    # fp8 fc1 operands in DoubleRow layout [128, KC, *]:
    # element (p, j, m) holds contraction index k = 128*j + p
    w1t = W1p.T.astype(FP8)  # [D_IN, H1]
    w1q_h = np.ascontiguousarray(
        w1t.reshape(KC, 128, H1).transpose(1, 0, 2))
    xt = x.T.astype(FP8)  # [D_IN, B]
    xq_h = np.ascontiguousarray(
        xt.reshape(KC, 128, B).transpose(1, 0, 2))

    b1c_h = b1p.reshape(CC, 128).T
    # chunk c: s = c//NT, tile t = c%NT, partition k <-> group 128*t + k
    w2c_h = np.empty((128, CC), dtype=np.float32)
    for c in range(CC):
        w2c_h[:, c] = W2[128 * (c % NT):128 * (c % NT) + 128, c // NT]
    # fc2 output is emitted as tanh((z+b2)/2) = sigmoid(z+b2) - 0.5 via
    # ACT(Tanh, scale=0.5, bias=b2/2); fc3 runs on W3' = 2*W3 in fp8
    # DoubleRow and the kernel applies out = 0.25*psum + b3' with
    # b3' = b3 + 0.5*W3.sum(1)
    b2c_h = (0.5 * b2).reshape(NT, 128).T
    b3p = b3 + 0.5 * W3.sum(axis=1)
    b3c_h = b3p.reshape(ND, 128).T
    cst_h = np.ascontiguousarray(
        np.concatenate([b1c_h, w2c_h, b2c_h, b3c_h], axis=1),
        dtype=np.float32)  # [128, 2*CC + NT + ND]
    w3t = (2.0 * W3).T.astype(FP8)  # [H2, D_OUT]
    w3q_h = np.ascontiguousarray(
        w3t.reshape(2, 2, 128, D_OUT).transpose(2, 0, 1, 3))

    in_maps = []
    for i in range(N_CORES):
        in_maps.append({
            "xq": np.ascontiguousarray(
                xq_h[:, :, i * B_SHARD:(i + 1) * B_SHARD]),
            "w1q": w1q_h,
            "cst": cst_h,
            "w3q": w3q_h,
        })
    return in_maps


def kernel(x, W1, b1, W2, b2, W3, b3):
    import os
    from concourse.bass_utils import run_bass_kernel_spmd

    nc = get_nc()
    in_maps = make_in_maps(x, W1, b1, W2, b2, W3, b3)
    # force tracing off for this call: the agent image lacks the axon NTFF
    # hook module, so a stray BASS_TRACE=1 would crash the run
    prev = os.environ.get("BASS_NEVER_TRACE")
    os.environ["BASS_NEVER_TRACE"] = "1"
    try:
        res = run_bass_kernel_spmd(nc, in_maps, core_ids=list(range(N_CORES)))
    finally:
        if prev is None:
            os.environ.pop("BASS_NEVER_TRACE", None)
        else:
            os.environ["BASS_NEVER_TRACE"] = prev
    outT = np.concatenate([res.results[i]["out"].astype(np.float32)
                           for i in range(N_CORES)], axis=1)  # [D_OUT, B]
    return np.ascontiguousarray(outT.T)

